# revision 2
# baseline (speedup 1.0000x reference)
"""Trainium2 Bass kernel for nn_CrossAttentionBlock_44289702756632 (v7).

The attention collapses exactly (uniform causal softmax over a time-repeated
key -> att @ V == V), so the block is LN1 -> +yv -> LN2 -> MLP with residual.

v4 (from v2 @ 372us):
  * scaled transposes: x3T = (x2c * rs2)^T is computed by ONE regular fp16
    matmul per (slice, d-chunk) with rhs = diag(rs2) built on GPSIMD via
    affine_select -> the x3h elementwise pass disappears from DVE.
  * LN2 stats: host centers yv (yc = yv - mean) so mean_d(x2c) == 0; var2 is
    a GPSIMD square + one DVE tensor_reduce (replaces bn_stats2 + bn_aggr2).
  * final residual: affine_then_add (x2c*rs2 + pout) -> fp16.
  * 2-stage software pipeline so the PE stream is [fc(g-2), ptrMM(g-1),
    pr(g-2)] -- the diag matmuls for group g-1 sit behind fc(g-2), giving the
    DVE/GPSIMD LN chain a full group period of slack (v2 stalled PE here).
  * all DMA (in + out) on the sync HWDGE ring; ACT does only gelu + psum
    copies.
"""

import numpy as np

B, A, T, D = 8, 128, 128, 256
DFF = 4 * D
N_CORES = 8
S_TOTAL = B * A
S_CORE = S_TOTAL // N_CORES   # 128 slices per core
G = 8                         # slices per group
NG = S_CORE // G              # 16 groups
EPS = 1e-5

_cache = {}


def _build(flags):
    import concourse.bass as bass
    import concourse.tile as tile
    from concourse import bacc, mybir
    from contextlib import ExitStack

    need_g1, need_g2, need_bfc, need_bpr = flags
    centered = not need_g1
    fast = centered and not need_g2
    f32 = mybir.dt.float32
    f16 = mybir.dt.float16
    AF = mybir.ActivationFunctionType
    OP = mybir.AluOpType
    AX = mybir.AxisListType

    nc = bacc.Bacc("TRN2", target_bir_lowering=False, debug=False)

    x_d = nc.dram_tensor("xg", [T, S_CORE, D], f16, kind="ExternalInput").ap()
    yv_d = nc.dram_tensor("yv", [S_CORE, D], f16, kind="ExternalInput").ap()
    wfc_d = nc.dram_tensor("wfc", [D, DFF], f16, kind="ExternalInput").ap()
    wpr_d = nc.dram_tensor("wpr", [DFF, D], f16, kind="ExternalInput").ap()
    if need_g1:
        g1_d = nc.dram_tensor("gam1", [D], f16, kind="ExternalInput").ap()
    if need_g2:
        g2_d = nc.dram_tensor("gam2", [D], f16, kind="ExternalInput").ap()
    if need_bfc:
        bfc_d = nc.dram_tensor("bfc", [DFF], f32, kind="ExternalInput").ap()
    if need_bpr:
        bpr_d = nc.dram_tensor("bpr", [D], f16, kind="ExternalInput").ap()
    out_d = nc.dram_tensor("out", [D, S_CORE, T], f16,
                           kind="ExternalOutput").ap()
    out_dr = out_d.rearrange("(k p) s t -> p k s t", k=2)

    def bcast(ap_2d, parts=128):
        return bass.AP(tensor=ap_2d.tensor, offset=ap_2d.offset,
                       ap=[[0, parts]] + [list(p) for p in ap_2d.ap])

    def fbcast(ap_col, n):
        # broadcast a [128, 1] SBUF column along a stride-0 free dim -> [128, n]
        return bass.AP(tensor=ap_col.tensor, offset=ap_col.offset,
                       ap=[list(ap_col.ap[0])] + [[0, n]])

    with tile.TileContext(nc) as tc, ExitStack() as ctx:
        consts = ctx.enter_context(tc.tile_pool(name="consts", bufs=1))
        xpool = ctx.enter_context(tc.tile_pool(name="xg", bufs=3))
        yvpool = ctx.enter_context(tc.tile_pool(name="yvb", bufs=3))
        spool = ctx.enter_context(tc.tile_pool(name="stats", bufs=3))
        sqpool = ctx.enter_context(tc.tile_pool(name="xsq", bufs=2))
        x2pool = ctx.enter_context(tc.tile_pool(name="x2", bufs=3))
        dgpool = ctx.enter_context(tc.tile_pool(name="diag", bufs=2))
        x3hpool = ctx.enter_context(tc.tile_pool(name="x3h", bufs=3))
        xtpool = ctx.enter_context(tc.tile_pool(name="x3T", bufs=2))
        htpool = ctx.enter_context(tc.tile_pool(name="hT", bufs=2))
        opool = ctx.enter_context(tc.tile_pool(name="outg", bufs=2))
        ptrpool = ctx.enter_context(tc.tile_pool(name="ptr", bufs=2, space="PSUM"))
        fcpool = ctx.enter_context(tc.tile_pool(name="pfc", bufs=4, space="PSUM"))
        prpool = ctx.enter_context(tc.tile_pool(name="ppr", bufs=1, space="PSUM"))

        # ---- constants ----
        wfc_sb = consts.tile([128, 2, DFF], f16)    # [d%128, d//128, f]
        nc.sync.dma_start(out=wfc_sb, in_=wfc_d.rearrange("(k p) f -> p k f", k=2))
        wpr_sb = consts.tile([128, 8, D], f16)      # [f%128, f//128, d]
        nc.sync.dma_start(out=wpr_sb, in_=wpr_d.rearrange("(m p) d -> p m d", m=8))
        if not fast:
            ident_sb = consts.tile([128, 128], f16)
            nc.gpsimd.memset(ident_sb, 1.0)
            nc.gpsimd.affine_select(
                out=ident_sb, in_=ident_sb, compare_op=OP.is_equal,
                fill=0.0, base=0, pattern=[[-1, 128]], channel_multiplier=1)
        if need_g1:
            g1_sb = consts.tile([128, D], f16)
            nc.gpsimd.dma_start(out=g1_sb, in_=bcast(g1_d[None, :]))
        if need_g2:
            g2_sb = consts.tile([128, 2], f16)
            nc.sync.dma_start(out=g2_sb, in_=g2_d.rearrange("(k p) -> p k", k=2))
        if need_bfc:
            bfc_sb = consts.tile([128, 8], f32)     # [f%128, f//128]
            nc.sync.dma_start(out=bfc_sb, in_=bfc_d.rearrange("(m p) -> p m", m=8))
        if need_bpr:
            bpr_sb = consts.tile([128, 2], f32)
            nc.sync.dma_start(out=bpr_sb, in_=bpr_d.rearrange("(k p) -> p k", k=2))

        # minimax quadratic rsqrt seed on v in [0.55, 1.6] (e0 ~1.6e-2);
        # one NR iteration brings it to ~4e-4 relative.
        RA, RB, RC = 1.92329830, -1.28668376, 0.36669836

        def nr_rsqrt(eng, var_ap, tag, iters=1):
            """rstd = 1/sqrt(var + EPS): quadratic seed + NR iteration(s)."""
            v = spool.tile([128, G], f32, tag="v" + tag)
            eng.tensor_scalar_add(v, var_ap, float(EPS))
            y = spool.tile([128, G], f32, tag="y" + tag)
            eng.tensor_scalar(out=y, in0=v, scalar1=RC, scalar2=RB,
                              op0=OP.mult, op1=OP.add)
            eng.tensor_mul(y, y, v)
            eng.tensor_scalar_add(y, y, RA)
            w = spool.tile([128, G], f32, tag="w" + tag)
            for _ in range(iters):
                eng.tensor_mul(w, y, y)
                eng.tensor_mul(w, w, v)
                eng.tensor_scalar(out=w, in0=w, scalar1=-0.5, scalar2=1.5,
                                  op0=OP.mult, op1=OP.add)
                eng.tensor_mul(y, y, w)
            return y

        def front_dve(g):
            """DMA in + LN1 apply + LN2 scale factors (+ GPSIMD diag build)."""
            sl = slice(g * G, (g + 1) * G)
            xg = xpool.tile([128, G, D], f16)
            nc.sync.dma_start(out=xg, in_=x_d[:, sl, :])
            ycb = yvpool.tile([128, G, D], f16)
            nc.gpsimd.dma_start(out=ycb, in_=bcast(yv_d[sl, :]))

            # LN1 stats
            st1 = spool.tile([128, G, 6], f32, tag="st1")
            for s in range(G):
                nc.vector.bn_stats(st1[:, s, :], xg[:, s, :])
            mv1 = spool.tile([128, G, 2], f32, tag="mv1")
            for s in range(G):
                nc.vector.bn_aggr(mv1[:, s, :], st1[:, s, :])
            rs1 = nr_rsqrt(nc.vector, mv1[:, :, 1], "1")
            nb1 = spool.tile([128, G], f32, tag="nb1")
            nc.vector.tensor_mul(nb1, mv1[:, :, 0], rs1)
            nc.vector.tensor_scalar_mul(nb1, nb1, -1.0)

            # x2c = (x*rs1 + nb1)[*g1] + yc
            x2c = x2pool.tile([128, G, D], f16)
            if need_g1:
                for s in range(G):
                    nc.vector.tensor_scalar(
                        out=x2c[:, s, :], in0=xg[:, s, :],
                        scalar1=mv1[:, s, 0:1], scalar2=rs1[:, s:s + 1],
                        op0=OP.subtract, op1=OP.mult)
                    nc.vector.tensor_mul(x2c[:, s, :], x2c[:, s, :], g1_sb)
                    nc.vector.tensor_add(x2c[:, s, :], x2c[:, s, :],
                                         ycb[:, s, :])
            else:
                for s in range(G):
                    nc.vector.affine_then_add(
                        x2c[:, s, :], xg[:, s, :], ycb[:, s, :],
                        scale=rs1[:, s:s + 1], bias=nb1[:, s:s + 1])

            # LN2 variance: GPSIMD square + one DVE reduce (mean==0 centered)
            x2sq = sqpool.tile([128, G, D], f16, tag="x2sq")
            nc.gpsimd.tensor_mul(x2sq, x2c, x2c)
            ev2 = spool.tile([128, G], f32, tag="ev2")
            nc.vector.tensor_reduce(ev2, x2sq, AX.X, OP.add)
            if centered:
                mean2 = None
                nc.gpsimd.tensor_scalar_mul(ev2, ev2, 1.0 / D)
                rs2 = nr_rsqrt(nc.gpsimd, ev2, "2")
            else:
                nc.vector.tensor_scalar_mul(ev2, ev2, 1.0 / D)
                sum2 = spool.tile([128, G], f32, tag="sum2")
                nc.vector.tensor_reduce(sum2, x2c, AX.X, OP.add)
                mean2 = spool.tile([128, G], f32, tag="mean2")
                nc.vector.tensor_scalar_mul(mean2, sum2, 1.0 / D)
                m2sq = spool.tile([128, G], f32, tag="m2sq")
                nc.vector.tensor_mul(m2sq, mean2, mean2)
                nc.vector.tensor_sub(ev2, ev2, m2sq)
                rs2 = nr_rsqrt(nc.vector, ev2, "2")

            if fast:
                # diag(rs2_s) tiles for the scaled-transpose matmuls
                rs2h = spool.tile([128, G], f16, tag="rs2h")
                nc.gpsimd.tensor_copy(rs2h, rs2)
                dg = dgpool.tile([128, G, 128], f16, tag="dg")
                rs2b = bass.AP(tensor=rs2h.tensor, offset=rs2h.offset,
                               ap=[list(p) for p in rs2h.ap] + [[0, 128]])
                nc.gpsimd.affine_select(
                    out=dg, in_=rs2b, compare_op=OP.is_equal,
                    fill=0.0, base=0, pattern=[[0, G], [-1, 128]],
                    channel_multiplier=1)
                x3h = None
            else:
                dg = None
                x3h = x3hpool.tile([128, G, D], f16)
                for s in range(G):
                    if centered:
                        nc.vector.tensor_scalar_mul(
                            x3h[:, s, :], x2c[:, s, :], rs2[:, s:s + 1])
                    else:
                        nc.vector.tensor_scalar(
                            out=x3h[:, s, :], in0=x2c[:, s, :],
                            scalar1=mean2[:, s:s + 1],
                            scalar2=rs2[:, s:s + 1],
                            op0=OP.subtract, op1=OP.mult)
            return dict(sl=sl, x2c=x2c, rs2=rs2, dg=dg, x3h=x3h)

        def front_pe(st):
            """scaled-transpose matmuls -> ptr psum -> x3T (ACT copies)."""
            x2c, dg, x3h = st["x2c"], st["dg"], st["x3h"]
            x3T = xtpool.tile([128, 2, G, 128], f16)
            for q in range(4):
                ptr = ptrpool.tile([128, 4, 128], f32, tag="ptr")
                for i in range(4):
                    s = 2 * q + i % 2
                    k = i // 2
                    if fast:
                        nc.tensor.matmul(
                            ptr[:, i, :], x2c[:, s, k * 128:(k + 1) * 128],
                            dg[:, s, :], start=True, stop=True)
                    else:
                        nc.tensor.matmul(
                            ptr[:, i, :], x3h[:, s, k * 128:(k + 1) * 128],
                            ident_sb, start=True, stop=True)
                # ptr blocks are ordered [k, s-pair] per q: i = 2*k + sp
                nc.scalar.copy(
                    x3T[:, :, 2 * q:2 * q + 2, :],
                    ptr.rearrange("p (k sp) t -> p k sp t", k=2))
            st["x3T"] = x3T

        def back_fc(st):
            """fc matmuls + gelu -> hT fp16."""
            x3T = st["x3T"]
            hT = htpool.tile([128, 8, G * 128], f16)
            st["hT"] = hT
            for m in range(8):
                ms = slice(m * 128, (m + 1) * 128)
                for h in range(2):
                    ph = fcpool.tile([128, 512], f32, tag="ph")
                    for k in range(2):
                        nc.tensor.matmul(
                            ph, wfc_sb[:, k, ms],
                            x3T[:, k, h * 4:(h + 1) * 4, :],
                            start=(k == 0), stop=(k == 1))
                    nc.scalar.activation(
                        hT[:, m, h * 512:(h + 1) * 512], ph, AF.Gelu,
                        bias=(bfc_sb[:, m:m + 1] if need_bfc else 0.0))

        def back_pr(st):
            """pr matmuls (transposed out) + residual + DMA out."""
            sl, hT = st["sl"], st["hT"]
            x3T = st["x3T"]
            outgT = opool.tile([128, 2, G, 128], f16)
            # quarter-bank pout tiles (1 PSUM bank each), reused across halves
            for h in range(2):
                hs = slice(h * 4, (h + 1) * 4)
                pq = [prpool.tile([128, 4, 128], f32, name="poT%d" % k,
                                  tag="poT%d" % k) for k in range(2)]
                for m in range(8):
                    for k in range(2):
                        nc.tensor.matmul(
                            pq[k], wpr_sb[:, m, k * 128:(k + 1) * 128],
                            hT[:, m, h * 512:(h + 1) * 512],
                            start=(m == 0), stop=(m == 7))
                for k in range(2):
                    if need_g2 or need_bpr:
                        nc.vector.affine_then_add(
                            outgT[:, k, hs, :], x3T[:, k, hs, :], pq[k],
                            scale=(g2_sb[:, k:k + 1] if need_g2 else 1.0),
                            bias=(bpr_sb[:, k:k + 1] if need_bpr else 0.0))
                    else:
                        nc.vector.tensor_add(outgT[:, k, hs, :],
                                             x3T[:, k, hs, :], pq[k])
            nc.sync.dma_start(out=out_dr[:, :, sl, :], in_=outgT)

        # ---- 2-stage software pipeline ----
        sts = [None] * NG
        for g in range(NG):
            sts[g] = front_dve(g)
            if g >= 2:
                back_fc(sts[g - 2])
            if g >= 1:
                front_pe(sts[g - 1])
            if g >= 2:
                back_pr(sts[g - 2])
        front_pe(sts[NG - 1])
        for g in (NG - 2, NG - 1):
            back_fc(sts[g])
            back_pr(sts[g])

    nc.compile()
    return nc


def _prepare(inputs):
    """Host-side preprocessing: fold the degenerate attention + biases."""
    x = np.asarray(inputs["x"], dtype=np.float32)
    cx = np.asarray(inputs["cx"], dtype=np.float32)
    wkv = np.asarray(inputs["wkv"], dtype=np.float32)
    bkv = np.asarray(inputs["bkv"], dtype=np.float32)
    wo = np.asarray(inputs["wo"], dtype=np.float32)
    bo = np.asarray(inputs["bo"], dtype=np.float32)
    w_fc = np.asarray(inputs["w_fc"], dtype=np.float32)
    b_fc = np.asarray(inputs["b_fc"], dtype=np.float32)
    w_pr = np.asarray(inputs["w_pr"], dtype=np.float32)
    b_pr = np.asarray(inputs["b_pr"], dtype=np.float32)
    ln1_w = np.asarray(inputs["ln1_w"], dtype=np.float32)
    ln1_b = np.asarray(inputs["ln1_b"], dtype=np.float32)
    ln2_w = np.asarray(inputs["ln2_w"], dtype=np.float32)
    ln2_b = np.asarray(inputs["ln2_b"], dtype=np.float32)

    wvo = wkv[:, D:] @ wo
    bvo = bkv[D:] @ wo + bo
    yv = cx.reshape(S_TOTAL, D) @ wvo + bvo
    yv = yv + ln1_b[None, :]                    # fold LN1 beta

    need_g1 = not np.all(ln1_w == 1.0)
    need_g2 = not np.all(ln2_w == 1.0)
    if not need_g1:
        # center yv: LN1 output is centered, so mean_d(x2c) == 0 exactly
        yv = yv - yv.mean(axis=-1, keepdims=True)
    wfc_eff = (ln2_w[:, None] * w_fc) if need_g2 else w_fc
    bfc_eff = b_fc + ln2_b @ w_fc
    bpr_eff = b_pr + ln2_b
    need_bfc = not np.all(bfc_eff == 0.0)
    need_bpr = not np.all(bpr_eff == 0.0)

    flags = (need_g1, need_g2, need_bfc, need_bpr)
    x_flat = x.reshape(S_TOTAL, T, D)

    in_maps = []
    for c in range(N_CORES):
        xc = x_flat[c * S_CORE:(c + 1) * S_CORE]          # [S, T, D]
        xc = np.ascontiguousarray(
            xc.transpose(1, 0, 2).astype(np.float16))     # [T, S, D] fp16
        m = {
            "xg": xc,
            "yv": np.ascontiguousarray(
                yv[c * S_CORE:(c + 1) * S_CORE].astype(np.float16)),
            "wfc": np.ascontiguousarray(wfc_eff.astype(np.float16)),
            "wpr": np.ascontiguousarray(w_pr.astype(np.float16)),
        }
        if need_g1:
            m["gam1"] = ln1_w.astype(np.float16)
        if need_g2:
            m["gam2"] = ln2_w.astype(np.float16)
        if need_bfc:
            m["bfc"] = bfc_eff.astype(np.float32)
        if need_bpr:
            m["bpr"] = bpr_eff.astype(np.float16)
        in_maps.append(m)
    return flags, in_maps


def run(inputs, trace=False):
    from concourse.bass_utils import run_bass_kernel_spmd

    flags, in_maps = _prepare(inputs)
    if flags not in _cache:
        _cache[flags] = _build(flags)
    nc = _cache[flags]
    res = run_bass_kernel_spmd(nc, in_maps, list(range(N_CORES)), trace=trace)
    outs = []
    for c in range(N_CORES):
        o = res.results[c]["out"]                         # [D, S, T] fp16
        outs.append(o.transpose(1, 2, 0).astype(np.float32))
    out = np.concatenate(outs, axis=0)
    return out.reshape(B, A, T, D), res


def kernel(**inputs):
    out, _ = run(inputs, trace=False)
    return out


# revision 3
# speedup vs baseline: 1.0231x; 1.0231x over previous
"""Trainium2 Bass kernel for nn_CrossAttentionBlock_44289702756632 (v9).

The attention collapses exactly (uniform causal softmax over a time-repeated
key -> att @ V == V), so the block is LN1 -> +yv -> LN2 -> MLP with residual.

v4 (from v2 @ 372us):
  * scaled transposes: x3T = (x2c * rs2)^T is computed by ONE regular fp16
    matmul per (slice, d-chunk) with rhs = diag(rs2) built on GPSIMD via
    affine_select -> the x3h elementwise pass disappears from DVE.
  * LN2 stats: host centers yv (yc = yv - mean) so mean_d(x2c) == 0; var2 is
    a GPSIMD square + one DVE tensor_reduce (replaces bn_stats2 + bn_aggr2).
  * final residual: affine_then_add (x2c*rs2 + pout) -> fp16.
  * 2-stage software pipeline so the PE stream is [fc(g-2), ptrMM(g-1),
    pr(g-2)] -- the diag matmuls for group g-1 sit behind fc(g-2), giving the
    DVE/GPSIMD LN chain a full group period of slack (v2 stalled PE here).
  * all DMA (in + out) on the sync HWDGE ring; ACT does only gelu + psum
    copies.
"""

import numpy as np

B, A, T, D = 8, 128, 128, 256
DFF = 4 * D
N_CORES = 8
S_TOTAL = B * A
S_CORE = S_TOTAL // N_CORES   # 128 slices per core
G = 8                         # slices per group
NG = S_CORE // G              # 16 groups
EPS = 1e-5

_cache = {}


def _build(flags):
    import concourse.bass as bass
    import concourse.tile as tile
    from concourse import bacc, mybir
    from contextlib import ExitStack

    need_g1, need_g2, need_bfc, need_bpr = flags
    centered = not need_g1
    fast = centered and not need_g2
    f32 = mybir.dt.float32
    f16 = mybir.dt.float16
    AF = mybir.ActivationFunctionType
    OP = mybir.AluOpType
    AX = mybir.AxisListType

    nc = bacc.Bacc("TRN2", target_bir_lowering=False, debug=False)

    x_d = nc.dram_tensor("xg", [T, S_CORE, D], f16, kind="ExternalInput").ap()
    yv_d = nc.dram_tensor("yv", [S_CORE, D], f16, kind="ExternalInput").ap()
    wfc_d = nc.dram_tensor("wfc", [D, DFF], f16, kind="ExternalInput").ap()
    wpr_d = nc.dram_tensor("wpr", [DFF, D], f16, kind="ExternalInput").ap()
    if need_g1:
        g1_d = nc.dram_tensor("gam1", [D], f16, kind="ExternalInput").ap()
    if need_g2:
        g2_d = nc.dram_tensor("gam2", [D], f16, kind="ExternalInput").ap()
    if need_bfc:
        bfc_d = nc.dram_tensor("bfc", [DFF], f32, kind="ExternalInput").ap()
    if need_bpr:
        bpr_d = nc.dram_tensor("bpr", [D], f16, kind="ExternalInput").ap()
    out_d = nc.dram_tensor("out", [D, S_CORE, T], f16,
                           kind="ExternalOutput").ap()
    out_dr = out_d.rearrange("(k p) s t -> p k s t", k=2)

    def bcast(ap_2d, parts=128):
        return bass.AP(tensor=ap_2d.tensor, offset=ap_2d.offset,
                       ap=[[0, parts]] + [list(p) for p in ap_2d.ap])

    def fbcast(ap_col, n):
        # broadcast a [128, 1] SBUF column along a stride-0 free dim -> [128, n]
        return bass.AP(tensor=ap_col.tensor, offset=ap_col.offset,
                       ap=[list(ap_col.ap[0])] + [[0, n]])

    with tile.TileContext(nc) as tc, ExitStack() as ctx:
        consts = ctx.enter_context(tc.tile_pool(name="consts", bufs=1))
        xpool = ctx.enter_context(tc.tile_pool(name="xg", bufs=3))
        yvpool = ctx.enter_context(tc.tile_pool(name="yvb", bufs=3))
        spool = ctx.enter_context(tc.tile_pool(name="stats", bufs=3))
        sqpool = ctx.enter_context(tc.tile_pool(name="xsq", bufs=2))
        x2pool = ctx.enter_context(tc.tile_pool(name="x2", bufs=3))
        dgpool = ctx.enter_context(tc.tile_pool(name="diag", bufs=2))
        x3hpool = ctx.enter_context(tc.tile_pool(name="x3h", bufs=3))
        xtpool = ctx.enter_context(tc.tile_pool(name="x3T", bufs=2))
        htpool = ctx.enter_context(tc.tile_pool(name="hT", bufs=2))
        opool = ctx.enter_context(tc.tile_pool(name="outg", bufs=2))
        ptrpool = ctx.enter_context(tc.tile_pool(name="ptr", bufs=2, space="PSUM"))
        fcpool = ctx.enter_context(tc.tile_pool(name="pfc", bufs=4, space="PSUM"))
        prpool = ctx.enter_context(tc.tile_pool(name="ppr", bufs=1, space="PSUM"))

        # ---- constants ----
        wfc_sb = consts.tile([128, 2, DFF], f16)    # [d%128, d//128, f]
        nc.sync.dma_start(out=wfc_sb, in_=wfc_d.rearrange("(k p) f -> p k f", k=2))
        wpr_sb = consts.tile([128, 8, D], f16)      # [f%128, f//128, d]
        nc.sync.dma_start(out=wpr_sb, in_=wpr_d.rearrange("(m p) d -> p m d", m=8))
        if not fast:
            ident_sb = consts.tile([128, 128], f16)
            nc.gpsimd.memset(ident_sb, 1.0)
            nc.gpsimd.affine_select(
                out=ident_sb, in_=ident_sb, compare_op=OP.is_equal,
                fill=0.0, base=0, pattern=[[-1, 128]], channel_multiplier=1)
        if need_g1:
            g1_sb = consts.tile([128, D], f16)
            nc.gpsimd.dma_start(out=g1_sb, in_=bcast(g1_d[None, :]))
        if need_g2:
            g2_sb = consts.tile([128, 2], f16)
            nc.sync.dma_start(out=g2_sb, in_=g2_d.rearrange("(k p) -> p k", k=2))
        if need_bfc:
            bfc_sb = consts.tile([128, 8], f32)     # [f%128, f//128]
            nc.sync.dma_start(out=bfc_sb, in_=bfc_d.rearrange("(m p) -> p m", m=8))
        if need_bpr:
            bpr_sb = consts.tile([128, 2], f32)
            nc.sync.dma_start(out=bpr_sb, in_=bpr_d.rearrange("(k p) -> p k", k=2))

        # minimax quadratic rsqrt seed on v in [0.55, 1.6] (e0 ~1.6e-2);
        # one NR iteration brings it to ~4e-4 relative.
        RA, RB, RC = 1.92329830, -1.28668376, 0.36669836

        def nr_rsqrt(eng, var_ap, tag, iters=1):
            """rstd = 1/sqrt(var + EPS): quadratic seed + NR iteration(s)."""
            v = spool.tile([128, G], f32, tag="v" + tag)
            eng.tensor_scalar_add(v, var_ap, float(EPS))
            y = spool.tile([128, G], f32, tag="y" + tag)
            eng.tensor_scalar(out=y, in0=v, scalar1=RC, scalar2=RB,
                              op0=OP.mult, op1=OP.add)
            eng.tensor_mul(y, y, v)
            eng.tensor_scalar_add(y, y, RA)
            w = spool.tile([128, G], f32, tag="w" + tag)
            for _ in range(iters):
                eng.tensor_mul(w, y, y)
                eng.tensor_mul(w, w, v)
                eng.tensor_scalar(out=w, in0=w, scalar1=-0.5, scalar2=1.5,
                                  op0=OP.mult, op1=OP.add)
                eng.tensor_mul(y, y, w)
            return y

        def front_dve(g):
            """DMA in + LN1 apply + LN2 scale factors (+ GPSIMD diag build)."""
            sl = slice(g * G, (g + 1) * G)
            xg = xpool.tile([128, G, D], f16)
            nc.sync.dma_start(out=xg, in_=x_d[:, sl, :])
            ycb = yvpool.tile([128, G, D], f16)
            nc.gpsimd.dma_start(out=ycb, in_=bcast(yv_d[sl, :]))

            # LN1 stats
            st1 = spool.tile([128, G, 6], f32, tag="st1")
            for s in range(G):
                nc.vector.bn_stats(st1[:, s, :], xg[:, s, :])
            mv1 = spool.tile([128, G, 2], f32, tag="mv1")
            for s in range(G):
                nc.vector.bn_aggr(mv1[:, s, :], st1[:, s, :])
            rs1 = nr_rsqrt(nc.vector, mv1[:, :, 1], "1")
            nb1 = spool.tile([128, G], f32, tag="nb1")
            nc.vector.tensor_mul(nb1, mv1[:, :, 0], rs1)
            nc.vector.tensor_scalar_mul(nb1, nb1, -1.0)

            # x2c = (x*rs1 + nb1)[*g1] + yc
            x2c = x2pool.tile([128, G, D], f16)
            if need_g1:
                for s in range(G):
                    nc.vector.tensor_scalar(
                        out=x2c[:, s, :], in0=xg[:, s, :],
                        scalar1=mv1[:, s, 0:1], scalar2=rs1[:, s:s + 1],
                        op0=OP.subtract, op1=OP.mult)
                    nc.vector.tensor_mul(x2c[:, s, :], x2c[:, s, :], g1_sb)
                    nc.vector.tensor_add(x2c[:, s, :], x2c[:, s, :],
                                         ycb[:, s, :])
            else:
                for s in range(G):
                    nc.vector.affine_then_add(
                        x2c[:, s, :], xg[:, s, :], ycb[:, s, :],
                        scale=rs1[:, s:s + 1], bias=nb1[:, s:s + 1])

            # LN2 variance: GPSIMD square + one DVE reduce (mean==0 centered)
            x2sq = sqpool.tile([128, G, D], f16, tag="x2sq")
            nc.gpsimd.tensor_mul(x2sq, x2c, x2c)
            ev2 = spool.tile([128, G], f32, tag="ev2")
            nc.vector.tensor_reduce(ev2, x2sq, AX.X, OP.add)
            if centered:
                mean2 = None
                nc.gpsimd.tensor_scalar_mul(ev2, ev2, 1.0 / D)
                rs2 = nr_rsqrt(nc.gpsimd, ev2, "2")
            else:
                nc.vector.tensor_scalar_mul(ev2, ev2, 1.0 / D)
                sum2 = spool.tile([128, G], f32, tag="sum2")
                nc.vector.tensor_reduce(sum2, x2c, AX.X, OP.add)
                mean2 = spool.tile([128, G], f32, tag="mean2")
                nc.vector.tensor_scalar_mul(mean2, sum2, 1.0 / D)
                m2sq = spool.tile([128, G], f32, tag="m2sq")
                nc.vector.tensor_mul(m2sq, mean2, mean2)
                nc.vector.tensor_sub(ev2, ev2, m2sq)
                rs2 = nr_rsqrt(nc.vector, ev2, "2")

            if fast:
                # diag(rs2_s) tiles for the scaled-transpose matmuls
                rs2h = spool.tile([128, G], f16, tag="rs2h")
                nc.gpsimd.tensor_copy(rs2h, rs2)
                dg = dgpool.tile([128, G, 128], f16, tag="dg")
                rs2b = bass.AP(tensor=rs2h.tensor, offset=rs2h.offset,
                               ap=[list(p) for p in rs2h.ap] + [[0, 128]])
                nc.gpsimd.affine_select(
                    out=dg, in_=rs2b, compare_op=OP.is_equal,
                    fill=0.0, base=0, pattern=[[0, G], [-1, 128]],
                    channel_multiplier=1)
                x3h = None
            else:
                dg = None
                x3h = x3hpool.tile([128, G, D], f16)
                for s in range(G):
                    if centered:
                        nc.vector.tensor_scalar_mul(
                            x3h[:, s, :], x2c[:, s, :], rs2[:, s:s + 1])
                    else:
                        nc.vector.tensor_scalar(
                            out=x3h[:, s, :], in0=x2c[:, s, :],
                            scalar1=mean2[:, s:s + 1],
                            scalar2=rs2[:, s:s + 1],
                            op0=OP.subtract, op1=OP.mult)
            return dict(sl=sl, x2c=x2c, rs2=rs2, dg=dg, x3h=x3h)

        def front_pe(st):
            """scaled-transpose matmuls -> ptr psum -> x3T (ACT copies)."""
            x2c, dg, x3h = st["x2c"], st["dg"], st["x3h"]
            x3T = xtpool.tile([128, 2, G, 128], f16)
            for q in range(4):
                ptr = ptrpool.tile([128, 4, 128], f32, tag="ptr")
                for i in range(4):
                    s = 2 * q + i % 2
                    k = i // 2
                    if fast:
                        nc.tensor.matmul(
                            ptr[:, i, :], x2c[:, s, k * 128:(k + 1) * 128],
                            dg[:, s, :], start=True, stop=True)
                    else:
                        nc.tensor.matmul(
                            ptr[:, i, :], x3h[:, s, k * 128:(k + 1) * 128],
                            ident_sb, start=True, stop=True)
                # ptr blocks are ordered [k, s-pair] per q: i = 2*k + sp
                nc.scalar.copy(
                    x3T[:, :, 2 * q:2 * q + 2, :],
                    ptr.rearrange("p (k sp) t -> p k sp t", k=2))
            st["x3T"] = x3T

        def back_fc(st):
            """fc matmuls + gelu -> hT fp16."""
            x3T = st["x3T"]
            hT = htpool.tile([128, 8, G * 128], f16)
            st["hT"] = hT
            for m in range(8):
                ms = slice(m * 128, (m + 1) * 128)
                for h in range(2):
                    ph = fcpool.tile([128, 512], f32, tag="ph")
                    for k in range(2):
                        nc.tensor.matmul(
                            ph, wfc_sb[:, k, ms],
                            x3T[:, k, h * 4:(h + 1) * 4, :],
                            start=(k == 0), stop=(k == 1))
                    nc.scalar.activation(
                        hT[:, m, h * 512:(h + 1) * 512], ph, AF.Gelu,
                        bias=(bfc_sb[:, m:m + 1] if need_bfc else 0.0))

        def back_pr(st):
            """pr matmuls (transposed out) + residual + DMA out."""
            sl, hT = st["sl"], st["hT"]
            x3T = st["x3T"]
            outgT = opool.tile([128, 2, G, 128], f16)
            # quarter-bank pout tiles (1 PSUM bank each), reused across halves
            for h in range(2):
                hs = slice(h * 4, (h + 1) * 4)
                pq = [prpool.tile([128, 4, 128], f32, name="poT%d" % k,
                                  tag="poT%d" % k) for k in range(2)]
                for m in range(8):
                    for k in range(2):
                        nc.tensor.matmul(
                            pq[k], wpr_sb[:, m, k * 128:(k + 1) * 128],
                            hT[:, m, h * 512:(h + 1) * 512],
                            start=(m == 0), stop=(m == 7))
                for k in range(2):
                    if need_g2 or need_bpr:
                        nc.vector.affine_then_add(
                            outgT[:, k, hs, :], x3T[:, k, hs, :], pq[k],
                            scale=(g2_sb[:, k:k + 1] if need_g2 else 1.0),
                            bias=(bpr_sb[:, k:k + 1] if need_bpr else 0.0))
                    else:
                        nc.vector.tensor_add(outgT[:, k, hs, :],
                                             x3T[:, k, hs, :], pq[k])
            nc.sync.dma_start(out=out_dr[:, :, sl, :], in_=outgT)

        # ---- PE warm-up spanning the pipeline fill (~19us) ----
        # Keeps the PE busy (and HAM un-throttled) until group 0's scaled
        # transposes are ready; the dummies write a rotating ptr-pool bank.
        for w in range(88):
            pw = ptrpool.tile([128, 4, 128], f32, tag="ptr", name="pw%d" % w)
            nc.tensor.matmul(pw, wfc_sb[:, 0, 0:128], wfc_sb[:, 1, 0:512],
                             start=True, stop=True)

        # ---- 2-stage software pipeline ----
        sts = [None] * NG
        for g in range(NG):
            sts[g] = front_dve(g)
            if g >= 2:
                back_fc(sts[g - 2])
            if g >= 1:
                front_pe(sts[g - 1])
            if g >= 2:
                back_pr(sts[g - 2])
        front_pe(sts[NG - 1])
        for g in (NG - 2, NG - 1):
            back_fc(sts[g])
            back_pr(sts[g])

    nc.compile()
    return nc


def _prepare(inputs):
    """Host-side preprocessing: fold the degenerate attention + biases."""
    x = np.asarray(inputs["x"], dtype=np.float32)
    cx = np.asarray(inputs["cx"], dtype=np.float32)
    wkv = np.asarray(inputs["wkv"], dtype=np.float32)
    bkv = np.asarray(inputs["bkv"], dtype=np.float32)
    wo = np.asarray(inputs["wo"], dtype=np.float32)
    bo = np.asarray(inputs["bo"], dtype=np.float32)
    w_fc = np.asarray(inputs["w_fc"], dtype=np.float32)
    b_fc = np.asarray(inputs["b_fc"], dtype=np.float32)
    w_pr = np.asarray(inputs["w_pr"], dtype=np.float32)
    b_pr = np.asarray(inputs["b_pr"], dtype=np.float32)
    ln1_w = np.asarray(inputs["ln1_w"], dtype=np.float32)
    ln1_b = np.asarray(inputs["ln1_b"], dtype=np.float32)
    ln2_w = np.asarray(inputs["ln2_w"], dtype=np.float32)
    ln2_b = np.asarray(inputs["ln2_b"], dtype=np.float32)

    wvo = wkv[:, D:] @ wo
    bvo = bkv[D:] @ wo + bo
    yv = cx.reshape(S_TOTAL, D) @ wvo + bvo
    yv = yv + ln1_b[None, :]                    # fold LN1 beta

    need_g1 = not np.all(ln1_w == 1.0)
    need_g2 = not np.all(ln2_w == 1.0)
    if not need_g1:
        # center yv: LN1 output is centered, so mean_d(x2c) == 0 exactly
        yv = yv - yv.mean(axis=-1, keepdims=True)
    wfc_eff = (ln2_w[:, None] * w_fc) if need_g2 else w_fc
    bfc_eff = b_fc + ln2_b @ w_fc
    bpr_eff = b_pr + ln2_b
    need_bfc = not np.all(bfc_eff == 0.0)
    need_bpr = not np.all(bpr_eff == 0.0)

    flags = (need_g1, need_g2, need_bfc, need_bpr)
    x_flat = x.reshape(S_TOTAL, T, D)

    in_maps = []
    for c in range(N_CORES):
        xc = x_flat[c * S_CORE:(c + 1) * S_CORE]          # [S, T, D]
        xc = np.ascontiguousarray(
            xc.transpose(1, 0, 2).astype(np.float16))     # [T, S, D] fp16
        m = {
            "xg": xc,
            "yv": np.ascontiguousarray(
                yv[c * S_CORE:(c + 1) * S_CORE].astype(np.float16)),
            "wfc": np.ascontiguousarray(wfc_eff.astype(np.float16)),
            "wpr": np.ascontiguousarray(w_pr.astype(np.float16)),
        }
        if need_g1:
            m["gam1"] = ln1_w.astype(np.float16)
        if need_g2:
            m["gam2"] = ln2_w.astype(np.float16)
        if need_bfc:
            m["bfc"] = bfc_eff.astype(np.float32)
        if need_bpr:
            m["bpr"] = bpr_eff.astype(np.float16)
        in_maps.append(m)
    return flags, in_maps


def run(inputs, trace=False):
    from concourse.bass_utils import run_bass_kernel_spmd

    flags, in_maps = _prepare(inputs)
    if flags not in _cache:
        _cache[flags] = _build(flags)
    nc = _cache[flags]
    res = run_bass_kernel_spmd(nc, in_maps, list(range(N_CORES)), trace=trace)
    outs = []
    for c in range(N_CORES):
        o = res.results[c]["out"]                         # [D, S, T] fp16
        outs.append(o.transpose(1, 2, 0).astype(np.float32))
    out = np.concatenate(outs, axis=0)
    return out.reshape(B, A, T, D), res


def kernel(**inputs):
    out, _ = run(inputs, trace=False)
    return out
